# revision 14
# baseline (speedup 1.0000x reference)
"""Trainium2 Bass kernel for nn_Bspline_segment_calc.

Math: the reference builds a FIXED uniform extended grid (the `grid` input is
unused): knots g_i = -1.6 + 0.2*i.  With u = 5*x + 8 (x in [0,1) => u in
[8,13)), out[a, r, n] = M4(u - r) for r = 5..12, rows 0..4 identically zero.

Per element exactly FOUR rows are nonzero (cubic B-spline support): with
s = floor(5x) in {0..4} and t = frac(5x) in [0,1), rows 5+s..8+s carry the
four cardinal values

    v0 = (1-t)^3/6                      v1 = 0.5 t^3 - t^2 + 2/3
    v2 = -0.5 t^3 + 0.5 t^2 + 0.5 t + 1/6        v3 = t^3/6

with v0+v1+v2+v3 == 1 (partition of unity).  The device computes the three
independent planes v0, v2, v3; the host reconstructs v1 = 1-v0-v2-v3 and
places the four values at rows 5+s..8+s (pure linear assembly by the
host-derived segment index, exactly like the baseline's host-side zero rows).
All planes are C2-continuous in t, so fp16 rounding of t is harmless and a
knot-epsilon disagreement in s places near-identical values.

Device work per chunk: v2 is one fused 6-stage Horner custom DVE op; v0/v3
factor into an ACT Square (quadratic) times a linear term (one fp16 2x
tensor_tensor on DVE / one tensor_mul on GpSimd) -- three compute engines
run in parallel, each under the DMA wall.  The three plane-chunks are packed
side by side in ONE SBUF tile and leave in ONE wide HWDGE DMA per chunk
(lines up to 5.8 KB), so descriptor-gen stays off the critical path and the
software-DGE (gpsimd) queue is never used.

I/O per core: 0.61 MB in + 1.83 MB out (fp16) vs 11.25 MB for the dense
f32 8-row variant.

Layout: each core's [5, 62500] t-shard is flattened and padded to 128x2442
(128 partitions engages all 16 SDMA engines).  Output DRAM is chunk-major:
chunk c occupies [128, 3*w_c] contiguously; the host undoes the packing.
"""

import numpy as np

import concourse.bass as bass
import concourse.bacc as bacc
import concourse.tile as tile
from concourse import mybir
from concourse.bass_utils import run_bass_kernel_spmd
import concourse.dve_ops as dve_ops_mod
from concourse.dve_spec import (
    Spec, Src0, C0, C1, One, relu, sq, lower, _has_src1,
)
from concourse.dve_uop import DveOpSpec

N_CORES = 8
N_ROWS = 5          # x rows
N_BASIS = 13        # output basis rows (rows 0..4 are zero)
R_LO = 5            # first possibly-nonzero basis row
N_PLANES = 3        # device-computed value planes (v0, v2, v3)
N_FULL = 500000
N_SHARD = N_FULL // N_CORES          # 62500
N_ELEM = N_ROWS * N_SHARD            # 312500 elements per core
P = 128                              # SBUF partitions (all 16 DMA engines)
FD = -(-N_ELEM // P)                 # 2442 elements per partition
N_PAD = P * FD                       # 312576
C1V = float(np.float64(6.0) ** (-1.0 / 3.0))   # c with c^3 = 1/6
SQ6 = float(np.float64(6.0) ** (-1.0 / 2.0))   # s with s^2 = 1/6
N_CHUNKS = 4
FIRST_CHUNK = 256   # small first chunk => compute starts sooner
LAST_CHUNK = 192    # small last chunk => short un-overlapped tail DMA
GP_V3 = False       # v3 multiply on GpSimd (else DVE tensor_tensor)
V0_CUSTOM = True    # v0 as one fused DVE op (no q/q2, frees ACT+GpSimd)
Q_ENGINE = "gp"     # (1-t) on: "gp" | "dve" (4x ts) | "act" (Identity)
SKIP_INIT_BARRIER = True
ENABLE_ASSERTS = False
WBUFS = 6


def _chunks():
    lo, hi, n = 0, FD, N_CHUNKS
    bounds = [0]
    if FIRST_CHUNK and n > 1:
        bounds.append(FIRST_CHUNK)
        lo, n = FIRST_CHUNK, n - 1
    last = LAST_CHUNK if (LAST_CHUNK and n > 1) else 0
    mid_hi, mid_n = hi - last, n - (1 if last else 0)
    bounds += [lo + round(i * (mid_hi - lo) / mid_n) for i in range(1, mid_n + 1)]
    if last:
        bounds.append(hi)
    return list(zip(bounds[:-1], bounds[1:]))


def _register_dve_op(name, spec):
    for op in dve_ops_mod.OPS:
        if op.name == name:
            return op
    opcode = dve_ops_mod._CUSTOM_DVE_ROW_BASE + len(dve_ops_mod.OPS)
    assert opcode < 0x20, "custom DVE row overflow"
    shas = {}
    for ver in ("v3", "v4"):
        uops = lower(spec, ver=ver)
        shas[ver] = DveOpSpec(
            name=name, opcode=opcode, uops=uops, rd1_en=_has_src1(spec)
        ).sha(ver)
    op = dve_ops_mod.DveOp(name, spec, subdim=False, uops_sha=shas)
    dve_ops_mod.OPS.append(op)
    dve_ops_mod._SUB_OPCODE_FOR_NAME[name] = opcode
    dve_ops_mod.CUSTOM_DVE_SPECS[name] = spec
    return op


def _get_v2_op():
    # out = (((1-t)*s0)*t + s0)*t + s1   (6 stages; s0=0.5, s1=1/6)
    body = ((One - Src0) * C0 * Src0 + C0) * Src0 + C1
    spec = Spec(
        body=body,
        reference=lambda in0, in1, s0, s1, imm2: (
            (((np.float32(1.0) - in0.astype(np.float32)) * s0) * in0 + s0)
            * in0 + s1
        ).astype(np.float32),
    )
    return _register_dve_op("BSPLINE_V2_HORNER_ANT", spec)


def _get_v0_op():
    # out = relu(in0*s0 + s1)^3          (5 ALU stages)
    r = relu(Src0 * C0 + C1)
    spec = Spec(
        body=sq(r) * r,
        reference=lambda in0, in1, s0, s1, imm2: (
            np.maximum(in0 * s0 + s1, np.float32(0.0)).astype(np.float32) ** 3
        ).astype(np.float32),
    )
    return _register_dve_op("BSPLINE_EDGE_CUBE_ANT", spec)


def _register_const(nc, value):
    """Make `value` usable as an activation bias (const_aps lookup).
    Must be called inside the TileContext: the memset is tracked by Tile."""
    f32 = mybir.dt.float32
    key = (f32, float(value))
    if key in nc.const_aps.aps:
        return
    t = nc.alloc_sbuf_tensor(f"const-f32-{float(value)}", [128, 1], f32)
    nc.vector.memset(t.ap(), float(value))
    nc.const_aps.aps[key] = t.ap()


def _build_bass():
    v2_op = _get_v2_op()
    v0_op = _get_v0_op()
    f32 = mybir.dt.float32
    f16 = mybir.dt.float16
    if SKIP_INIT_BARRIER:
        # Skip Bass.__init__'s trailing all-engine barrier (only guards its
        # 0.0/1.0 const memsets; the earlier _nrt_pseudo_barrier already
        # orders the semaphore clears).  The only in-kernel reader of those
        # consts is the throwaway table-warm activation.  Saves ~2us.
        _orig_barrier = bass.Bass.all_engine_barrier
        bass.Bass.all_engine_barrier = lambda self: None
        try:
            nc = bacc.Bacc(
                "TRN2", target_bir_lowering=False, debug=False,
                num_devices=N_CORES, enable_asserts=ENABLE_ASSERTS,
            )
        finally:
            bass.Bass.all_engine_barrier = _orig_barrier
    else:
        nc = bacc.Bacc(
            "TRN2", target_bir_lowering=False, debug=False,
            num_devices=N_CORES, enable_asserts=ENABLE_ASSERTS,
        )
    t_dram = nc.dram_tensor("t", [N_PAD], f16, kind="ExternalInput")
    out_dram = nc.dram_tensor(
        "out", [N_PLANES * N_PAD], f16, kind="ExternalOutput"
    )
    tv = t_dram.ap().rearrange("(p f) -> p f", p=P)

    with tile.TileContext(nc) as tc:
        with (
            tc.tile_pool(name="const", bufs=1) as cpool,
            tc.tile_pool(name="work", bufs=WBUFS) as wpool,
        ):
            t_tile = cpool.tile([P, FD], f16, tag="t")
            for lo, hi in _chunks():
                nc.sync.dma_start(out=t_tile[:, lo:hi], in_=tv[:, lo:hi])

            # Warm the ACT table load before the input lands.
            warm = cpool.tile([P, 1], f32, tag="warm")
            nc.scalar.activation(
                warm[:], nc.const_aps.aps[(f32, 0.0)][:P, :],
                mybir.ActivationFunctionType.Square, bias=0.0, scale=1.0,
            )
            _register_const(nc, SQ6)   # bias for the (1-t) quadratic factor

            # ACT runs one chunk AHEAD of DVE (software pipelining): q3(c)
            # is already in SBUF when DVE reaches chunk c's tensor_tensor,
            # so the DVE stream never stalls and each chunk's combined
            # out-DMA fires as soon as its last DVE op retires.
            chunks = _chunks()
            q3_tiles = []
            for ci, (lo, hi) in enumerate(chunks):
                q3 = wpool.tile([P, hi - lo], f16, tag=f"q3_{ci}")
                nc.scalar.activation(
                    q3[:], t_tile[:, lo:hi],
                    mybir.ActivationFunctionType.Square, bias=0.0, scale=SQ6,
                )
                q3_tiles.append(q3)

            for ci, (lo, hi) in enumerate(chunks):
                ts = t_tile[:, lo:hi]
                w = hi - lo
                o_tile = wpool.tile([P, N_PLANES * w], f16, tag="o")
                o0, o2, o3 = o_tile[:, 0:w], o_tile[:, w:2 * w], o_tile[:, 2 * w:3 * w]

                q3 = q3_tiles[ci]
                nc.vector._custom_dve(
                    v2_op, out=o2, in0=ts, s0=0.5, s1=float(1.0 / 6.0),
                )
                # v3 = (t^2/6) * t
                if GP_V3:
                    nc.gpsimd.tensor_mul(o3, q3[:], ts)
                else:
                    nc.vector.tensor_tensor(o3, q3[:], ts, mybir.AluOpType.mult)
                if V0_CUSTOM:
                    # v0 = relu(-c*t + c)^3 = (1-t)^3/6, one fused DVE op
                    nc.vector._custom_dve(
                        v0_op, out=o0, in0=ts, s0=-C1V, s1=C1V,
                    )
                else:
                    # q = (1-t); q2 = (1-t)^2/6 on ACT; v0 = q2*q
                    q = wpool.tile([P, w], f16, tag="q")
                    if Q_ENGINE == "gp":
                        nc.gpsimd.tensor_scalar(
                            q[:], ts, -1.0, 1.0,
                            mybir.AluOpType.mult, mybir.AluOpType.add,
                        )
                    elif Q_ENGINE == "dve":
                        nc.vector.tensor_scalar(
                            q[:], ts, -1.0, 1.0,
                            mybir.AluOpType.mult, mybir.AluOpType.add,
                        )
                    else:
                        nc.scalar.activation(
                            q[:], ts, mybir.ActivationFunctionType.Identity,
                            bias=1.0, scale=-1.0,
                        )
                    q2 = wpool.tile([P, w], f16, tag="q2")
                    nc.scalar.activation(
                        q2[:], ts, mybir.ActivationFunctionType.Square,
                        bias=SQ6, scale=-SQ6,
                    )
                    nc.vector.tensor_tensor(o0, q2[:], q[:], mybir.AluOpType.mult)

                ov = out_dram.ap()[N_PLANES * P * lo : N_PLANES * P * hi]
                ov = ov.rearrange("(p f) -> p f", p=P)
                nc.sync.dma_start(out=ov, in_=o_tile[:])
    nc.compile()
    return nc


_NC_CACHE = None


def _get_nc():
    global _NC_CACHE
    if _NC_CACHE is None:
        _NC_CACHE = _build_bass()
    return _NC_CACHE


def make_shards(x):
    """Host prep: t = frac(5x) as fp16 shards (one per core) + segment index
    s = floor(5x) used for host-side placement of the four value planes."""
    xf = x.astype(np.float64)
    u = 5.0 * xf
    sf = np.floor(u)
    s = sf.astype(np.int16)                  # [5, N] in 0..4
    t = (u - sf).astype(np.float16)
    in_maps = []
    for i in range(N_CORES):
        sh = np.zeros(N_PAD, dtype=np.float16)
        sh[:N_ELEM] = np.ascontiguousarray(
            t[:, i * N_SHARD : (i + 1) * N_SHARD]
        ).reshape(-1)
        in_maps.append({"t": sh})
    return in_maps, s


def _decode_planes(o_flat):
    """Undo the chunk-major [128, 3*w] packing -> [3, N_PAD] fp32."""
    planes = np.empty((N_PLANES, P, FD), dtype=np.float32)
    for lo, hi in _chunks():
        seg = o_flat[N_PLANES * P * lo : N_PLANES * P * hi].reshape(
            P, N_PLANES, hi - lo
        )
        planes[:, :, lo:hi] = seg.transpose(1, 0, 2)
    return planes.reshape(N_PLANES, N_PAD)


def kernel(x, grid=None, k=None, **_ignored):
    x = np.asarray(x, dtype=np.float32)
    assert x.shape == (N_ROWS, N_FULL), x.shape
    nc = _get_nc()
    in_maps, s = make_shards(x)
    res = run_bass_kernel_spmd(nc, in_maps, list(range(N_CORES))).results
    vals = np.empty((N_ROWS, 4, N_FULL), dtype=np.float32)
    for i in range(N_CORES):
        o = _decode_planes(np.asarray(res[i]["out"]))   # [3, N_PAD] f32
        sl = slice(i * N_SHARD, (i + 1) * N_SHARD)
        v = o[:, :N_ELEM].reshape(N_PLANES, N_ROWS, N_SHARD)
        vals[:, 0, sl] = v[0]
        vals[:, 2, sl] = v[1]
        vals[:, 3, sl] = v[2]
    # partition of unity: v1 = 1 - v0 - v2 - v3 (linear host epilogue)
    vals[:, 1, :] = 1.0 - vals[:, 0, :] - vals[:, 2, :] - vals[:, 3, :]
    full = np.zeros((N_ROWS, N_BASIS, N_FULL), dtype=np.float32)
    idx = (R_LO + s.astype(np.int64))[:, None, :] + np.arange(4)[None, :, None]
    np.put_along_axis(full, idx, vals, axis=1)
    return full


# revision 18
# speedup vs baseline: 1.0351x; 1.0351x over previous
"""Trainium2 Bass kernel for nn_Bspline_segment_calc.

Math: the reference builds a FIXED uniform extended grid (the `grid` input is
unused): knots g_i = -1.6 + 0.2*i.  With u = 5*x + 8 (x in [0,1) => u in
[8,13)), out[a, r, n] = M4(u - r) for r = 5..12, rows 0..4 identically zero.

Per element exactly FOUR rows are nonzero (cubic B-spline support): with
s = floor(5x) in {0..4} and t = frac(5x) in [0,1), rows 5+s..8+s carry the
four cardinal values

    v0 = (1-t)^3/6                      v1 = 0.5 t^3 - t^2 + 2/3
    v2 = -0.5 t^3 + 0.5 t^2 + 0.5 t + 1/6        v3 = t^3/6

with v0+v1+v2+v3 == 1 (partition of unity).  The device computes the three
independent planes v0, v2, v3; the host reconstructs v1 = 1-v0-v2-v3 and
places the four values at rows 5+s..8+s (pure linear assembly by the
host-derived segment index, exactly like the baseline's host-side zero rows).
All planes are C2-continuous in t, so fp16 rounding of t is harmless and a
knot-epsilon disagreement in s places near-identical values.

Device work per chunk: v2 is one fused 6-stage Horner custom DVE op; v0/v3
factor into an ACT Square (quadratic) times a linear term (one fp16 2x
tensor_tensor on DVE / one tensor_mul on GpSimd) -- three compute engines
run in parallel, each under the DMA wall.  The three plane-chunks are packed
side by side in ONE SBUF tile and leave in ONE wide HWDGE DMA per chunk
(lines up to 5.8 KB), so descriptor-gen stays off the critical path and the
software-DGE (gpsimd) queue is never used.

I/O per core: 0.61 MB in + 1.83 MB out (fp16) vs 11.25 MB for the dense
f32 8-row variant.

Layout: each core's [5, 62500] t-shard is flattened and padded to 128x2442
(128 partitions engages all 16 SDMA engines).  Output DRAM is chunk-major:
chunk c occupies [128, 3*w_c] contiguously; the host undoes the packing.
"""

import numpy as np

import concourse.bass as bass
import concourse.bacc as bacc
import concourse.tile as tile
from concourse import mybir
from concourse.bass_utils import run_bass_kernel_spmd
import concourse.dve_ops as dve_ops_mod
from concourse.dve_spec import (
    Spec, Src0, C0, C1, One, relu, sq, lower, _has_src1,
)
from concourse.dve_uop import DveOpSpec

N_CORES = 8
N_ROWS = 5          # x rows
N_BASIS = 13        # output basis rows (rows 0..4 are zero)
R_LO = 5            # first possibly-nonzero basis row
N_PLANES = 3        # device-computed value planes (v0, v2, v3)
N_FULL = 500000
N_SHARD = N_FULL // N_CORES          # 62500
N_ELEM = N_ROWS * N_SHARD            # 312500 elements per core
P = 128                              # SBUF partitions (all 16 DMA engines)
FD = -(-N_ELEM // P)                 # 2442 elements per partition
N_PAD = P * FD                       # 312576
C1V = float(np.float64(6.0) ** (-1.0 / 3.0))   # c with c^3 = 1/6
SQ6 = float(np.float64(6.0) ** (-1.0 / 2.0))   # s with s^2 = 1/6
# Graduated compute/out chunks: tiny chunks at the head hide the first-DMA
# ramp (compute starts on 16 KB of data), big middle chunks amortize per-op
# init, a small tail chunk keeps the final un-overlapped DMA short.
OUT_BOUNDS = [0, 64, 256, 700, 1448, 2250, 2442]
IN_BOUNDS = [0, 64, 256, 1349, 2442]
GP_MIN_W = 400      # chunks at least this wide put the v3 multiply on GpSimd
SPLIT_IN0 = True    # first input DMA split across both HWDGE queues
GP_V3 = True        # v3 multiply on GpSimd (else DVE tensor_tensor)
V0_CUSTOM = True    # v0 as one fused DVE op (no q/q2, frees ACT+GpSimd)
Q_ENGINE = "gp"     # (1-t) on: "gp" | "dve" (4x ts) | "act" (Identity)
SKIP_INIT_BARRIER = True
ENABLE_ASSERTS = False
WBUFS = 6


def _out_chunks():
    return list(zip(OUT_BOUNDS[:-1], OUT_BOUNDS[1:]))


def _in_chunks():
    return list(zip(IN_BOUNDS[:-1], IN_BOUNDS[1:]))


def _register_dve_op(name, spec):
    for op in dve_ops_mod.OPS:
        if op.name == name:
            return op
    opcode = dve_ops_mod._CUSTOM_DVE_ROW_BASE + len(dve_ops_mod.OPS)
    assert opcode < 0x20, "custom DVE row overflow"
    shas = {}
    for ver in ("v3", "v4"):
        uops = lower(spec, ver=ver)
        shas[ver] = DveOpSpec(
            name=name, opcode=opcode, uops=uops, rd1_en=_has_src1(spec)
        ).sha(ver)
    op = dve_ops_mod.DveOp(name, spec, subdim=False, uops_sha=shas)
    dve_ops_mod.OPS.append(op)
    dve_ops_mod._SUB_OPCODE_FOR_NAME[name] = opcode
    dve_ops_mod.CUSTOM_DVE_SPECS[name] = spec
    return op


def _get_v2_op():
    # out = (((1-t)*s0)*t + s0)*t + s1   (6 stages; s0=0.5, s1=1/6)
    body = ((One - Src0) * C0 * Src0 + C0) * Src0 + C1
    spec = Spec(
        body=body,
        reference=lambda in0, in1, s0, s1, imm2: (
            (((np.float32(1.0) - in0.astype(np.float32)) * s0) * in0 + s0)
            * in0 + s1
        ).astype(np.float32),
    )
    return _register_dve_op("BSPLINE_V2_HORNER_ANT", spec)


def _get_v0_op():
    # out = relu(in0*s0 + s1)^3          (5 ALU stages)
    r = relu(Src0 * C0 + C1)
    spec = Spec(
        body=sq(r) * r,
        reference=lambda in0, in1, s0, s1, imm2: (
            np.maximum(in0 * s0 + s1, np.float32(0.0)).astype(np.float32) ** 3
        ).astype(np.float32),
    )
    return _register_dve_op("BSPLINE_EDGE_CUBE_ANT", spec)


def _register_const(nc, value):
    """Make `value` usable as an activation bias (const_aps lookup).
    Must be called inside the TileContext: the memset is tracked by Tile."""
    f32 = mybir.dt.float32
    key = (f32, float(value))
    if key in nc.const_aps.aps:
        return
    t = nc.alloc_sbuf_tensor(f"const-f32-{float(value)}", [128, 1], f32)
    nc.vector.memset(t.ap(), float(value))
    nc.const_aps.aps[key] = t.ap()


def _build_bass():
    v2_op = _get_v2_op()
    v0_op = _get_v0_op()
    f32 = mybir.dt.float32
    f16 = mybir.dt.float16
    if SKIP_INIT_BARRIER:
        # Skip Bass.__init__'s trailing all-engine barrier (only guards its
        # 0.0/1.0 const memsets; the earlier _nrt_pseudo_barrier already
        # orders the semaphore clears).  The only in-kernel reader of those
        # consts is the throwaway table-warm activation.  Saves ~2us.
        _orig_barrier = bass.Bass.all_engine_barrier
        bass.Bass.all_engine_barrier = lambda self: None
        try:
            nc = bacc.Bacc(
                "TRN2", target_bir_lowering=False, debug=False,
                num_devices=N_CORES, enable_asserts=ENABLE_ASSERTS,
            )
        finally:
            bass.Bass.all_engine_barrier = _orig_barrier
    else:
        nc = bacc.Bacc(
            "TRN2", target_bir_lowering=False, debug=False,
            num_devices=N_CORES, enable_asserts=ENABLE_ASSERTS,
        )
    t_dram = nc.dram_tensor("t", [N_PAD], f16, kind="ExternalInput")
    out_dram = nc.dram_tensor(
        "out", [N_PLANES * N_PAD], f16, kind="ExternalOutput"
    )
    tv = t_dram.ap().rearrange("(p f) -> p f", p=P)

    with tile.TileContext(nc) as tc:
        with (
            tc.tile_pool(name="const", bufs=1) as cpool,
            tc.tile_pool(name="work", bufs=WBUFS) as wpool,
        ):
            t_tile = cpool.tile([P, FD], f16, tag="t")
            # First input chunk split across BOTH HWDGE queues so both DMA
            # rings ramp in parallel and compute starts earliest; remaining
            # inputs go on sync ahead of the out-DMAs.
            ics = _in_chunks()
            (lo0, hi0) = ics[0]
            if SPLIT_IN0:
                nc.scalar.dma_start(out=t_tile[:64, lo0:hi0], in_=tv[:64, lo0:hi0])
                nc.sync.dma_start(out=t_tile[64:, lo0:hi0], in_=tv[64:, lo0:hi0])
            else:
                nc.scalar.dma_start(out=t_tile[:, lo0:hi0], in_=tv[:, lo0:hi0])
            for lo, hi in ics[1:]:
                nc.sync.dma_start(out=t_tile[:, lo:hi], in_=tv[:, lo:hi])

            # Warm the ACT table load before the input lands.
            warm = cpool.tile([P, 1], f32, tag="warm")
            nc.scalar.activation(
                warm[:], nc.const_aps.aps[(f32, 0.0)][:P, :],
                mybir.ActivationFunctionType.Square, bias=0.0, scale=1.0,
            )
            _register_const(nc, SQ6)   # bias for the (1-t) quadratic factor

            for ci, (lo, hi) in enumerate(_out_chunks()):
                ts = t_tile[:, lo:hi]
                w = hi - lo
                o_tile = wpool.tile([P, N_PLANES * w], f16, tag="o")
                o0, o2, o3 = o_tile[:, 0:w], o_tile[:, w:2 * w], o_tile[:, 2 * w:3 * w]

                # q3 = t^2/6 on ACT; v2 custom on DVE in parallel.
                on_gp = GP_V3 and w >= GP_MIN_W
                if on_gp:
                    q3 = wpool.tile([P, w], f16, tag="q3")
                    nc.scalar.activation(
                        q3[:], ts, mybir.ActivationFunctionType.Square,
                        bias=0.0, scale=SQ6,
                    )
                nc.vector._custom_dve(
                    v2_op, out=o2, in0=ts, s0=0.5, s1=float(1.0 / 6.0),
                )
                # v3 = (t^2/6) * t on GpSimd for wide chunks; narrow chunks
                # (pipeline head/tail) use a third custom op straight from t
                # so they carry no cross-engine dependency.
                if on_gp:
                    nc.gpsimd.tensor_mul(o3, q3[:], ts)
                else:
                    nc.vector._custom_dve(
                        v0_op, out=o3, in0=ts, s0=C1V, s1=0.0,
                    )
                if V0_CUSTOM:
                    # v0 = relu(-c*t + c)^3 = (1-t)^3/6, one fused DVE op
                    nc.vector._custom_dve(
                        v0_op, out=o0, in0=ts, s0=-C1V, s1=C1V,
                    )
                else:
                    # q = (1-t); q2 = (1-t)^2/6 on ACT; v0 = q2*q
                    q = wpool.tile([P, w], f16, tag="q")
                    if Q_ENGINE == "gp":
                        nc.gpsimd.tensor_scalar(
                            q[:], ts, -1.0, 1.0,
                            mybir.AluOpType.mult, mybir.AluOpType.add,
                        )
                    elif Q_ENGINE == "dve":
                        nc.vector.tensor_scalar(
                            q[:], ts, -1.0, 1.0,
                            mybir.AluOpType.mult, mybir.AluOpType.add,
                        )
                    else:
                        nc.scalar.activation(
                            q[:], ts, mybir.ActivationFunctionType.Identity,
                            bias=1.0, scale=-1.0,
                        )
                    q2 = wpool.tile([P, w], f16, tag="q2")
                    nc.scalar.activation(
                        q2[:], ts, mybir.ActivationFunctionType.Square,
                        bias=SQ6, scale=-SQ6,
                    )
                    nc.vector.tensor_tensor(o0, q2[:], q[:], mybir.AluOpType.mult)

                ov = out_dram.ap()[N_PLANES * P * lo : N_PLANES * P * hi]
                ov = ov.rearrange("(p f) -> p f", p=P)
                nc.sync.dma_start(out=ov, in_=o_tile[:])
    nc.compile()
    return nc


_NC_CACHE = None


def _get_nc():
    global _NC_CACHE
    if _NC_CACHE is None:
        _NC_CACHE = _build_bass()
    return _NC_CACHE


def make_shards(x):
    """Host prep: t = frac(5x) as fp16 shards (one per core) + segment index
    s = floor(5x) used for host-side placement of the four value planes."""
    xf = x.astype(np.float64)
    u = 5.0 * xf
    sf = np.floor(u)
    s = sf.astype(np.int16)                  # [5, N] in 0..4
    t = (u - sf).astype(np.float16)
    in_maps = []
    for i in range(N_CORES):
        sh = np.zeros(N_PAD, dtype=np.float16)
        sh[:N_ELEM] = np.ascontiguousarray(
            t[:, i * N_SHARD : (i + 1) * N_SHARD]
        ).reshape(-1)
        in_maps.append({"t": sh})
    return in_maps, s


def _decode_planes(o_flat):
    """Undo the chunk-major [128, 3*w] packing -> [3, N_PAD] fp32."""
    planes = np.empty((N_PLANES, P, FD), dtype=np.float32)
    for lo, hi in _out_chunks():
        seg = o_flat[N_PLANES * P * lo : N_PLANES * P * hi].reshape(
            P, N_PLANES, hi - lo
        )
        planes[:, :, lo:hi] = seg.transpose(1, 0, 2)
    return planes.reshape(N_PLANES, N_PAD)


def kernel(x, grid=None, k=None, **_ignored):
    x = np.asarray(x, dtype=np.float32)
    assert x.shape == (N_ROWS, N_FULL), x.shape
    nc = _get_nc()
    in_maps, s = make_shards(x)
    res = run_bass_kernel_spmd(nc, in_maps, list(range(N_CORES))).results
    vals = np.empty((N_ROWS, 4, N_FULL), dtype=np.float32)
    for i in range(N_CORES):
        o = _decode_planes(np.asarray(res[i]["out"]))   # [3, N_PAD] f32
        sl = slice(i * N_SHARD, (i + 1) * N_SHARD)
        v = o[:, :N_ELEM].reshape(N_PLANES, N_ROWS, N_SHARD)
        vals[:, 0, sl] = v[0]
        vals[:, 2, sl] = v[1]
        vals[:, 3, sl] = v[2]
    # partition of unity: v1 = 1 - v0 - v2 - v3 (linear host epilogue)
    vals[:, 1, :] = 1.0 - vals[:, 0, :] - vals[:, 2, :] - vals[:, 3, :]
    full = np.zeros((N_ROWS, N_BASIS, N_FULL), dtype=np.float32)
    idx = (R_LO + s.astype(np.int64))[:, None, :] + np.arange(4)[None, :, None]
    np.put_along_axis(full, idx, vals, axis=1)
    return full


# revision 19
# speedup vs baseline: 1.0811x; 1.0444x over previous
"""Trainium2 Bass kernel for nn_Bspline_segment_calc.

Math: the reference builds a FIXED uniform extended grid (the `grid` input is
unused): knots g_i = -1.6 + 0.2*i.  With u = 5*x + 8 (x in [0,1) => u in
[8,13)), out[a, r, n] = M4(u - r) for r = 5..12, rows 0..4 identically zero.

Per element exactly FOUR rows are nonzero (cubic B-spline support): with
s = floor(5x) in {0..4} and t = frac(5x) in [0,1), rows 5+s..8+s carry the
four cardinal values

    v0 = (1-t)^3/6                      v1 = 0.5 t^3 - t^2 + 2/3
    v2 = -0.5 t^3 + 0.5 t^2 + 0.5 t + 1/6        v3 = t^3/6

with v0+v1+v2+v3 == 1 (partition of unity).  The device computes the three
independent planes v0, v2, v3; the host reconstructs v1 = 1-v0-v2-v3 and
places the four values at rows 5+s..8+s (pure linear assembly by the
host-derived segment index, exactly like the baseline's host-side zero rows).
All planes are C2-continuous in t, so fp16 rounding of t is harmless and a
knot-epsilon disagreement in s places near-identical values.

Device work per chunk: v2 is one fused 6-stage Horner custom DVE op; v0/v3
factor into an ACT Square (quadratic) times a linear term (one fp16 2x
tensor_tensor on DVE / one tensor_mul on GpSimd) -- three compute engines
run in parallel, each under the DMA wall.  The three plane-chunks are packed
side by side in ONE SBUF tile and leave in ONE wide HWDGE DMA per chunk
(lines up to 5.8 KB), so descriptor-gen stays off the critical path and the
software-DGE (gpsimd) queue is never used.

I/O per core: 0.61 MB in + 1.83 MB out (fp16) vs 11.25 MB for the dense
f32 8-row variant.

Layout: each core's [5, 62500] t-shard is flattened and padded to 128x2442
(128 partitions engages all 16 SDMA engines).  Output DRAM is chunk-major:
chunk c occupies [128, 3*w_c] contiguously; the host undoes the packing.
"""

import numpy as np

import concourse.bass as bass
import concourse.bacc as bacc
import concourse.tile as tile
from concourse import mybir
from concourse.bass_utils import run_bass_kernel_spmd
import concourse.dve_ops as dve_ops_mod
from concourse.dve_spec import (
    Spec, Src0, C0, C1, One, relu, sq, lower, _has_src1,
)
from concourse.dve_uop import DveOpSpec

N_CORES = 8
N_ROWS = 5          # x rows
N_BASIS = 13        # output basis rows (rows 0..4 are zero)
R_LO = 5            # first possibly-nonzero basis row
N_PLANES = 3        # device-computed value planes (v0, v2, v3)
N_FULL = 500000
N_SHARD = N_FULL // N_CORES          # 62500
N_ELEM = N_ROWS * N_SHARD            # 312500 elements per core
P = 128                              # SBUF partitions (all 16 DMA engines)
FD = -(-N_ELEM // P)                 # 2442 elements per partition
N_PAD = P * FD                       # 312576
C1V = float(np.float64(6.0) ** (-1.0 / 3.0))   # c with c^3 = 1/6
SQ6 = float(np.float64(6.0) ** (-1.0 / 2.0))   # s with s^2 = 1/6
N_OUT_CHUNKS = 6
FIRST_CHUNK = 256   # small first chunk => compute starts sooner
LAST_CHUNK = 192    # small last chunk => short un-overlapped tail DMA
N_IN_CHUNKS = 3     # input DMA granularity (decoupled from compute/out)
GP_V3 = True        # v3 multiply on GpSimd (else DVE tensor_tensor)
V0_CUSTOM = True    # v0 as one fused DVE op (no q/q2, frees ACT+GpSimd)
Q_ENGINE = "gp"     # (1-t) on: "gp" | "dve" (4x ts) | "act" (Identity)
SKIP_INIT_BARRIER = True
ENABLE_ASSERTS = False
WBUFS = 6


def _split(lo, hi, n, first=0, last=0):
    bounds = [lo]
    if first and n > 1:
        bounds.append(lo + first)
        lo, n = lo + first, n - 1
    last = last if (last and n > 1) else 0
    mid_hi, mid_n = hi - last, n - (1 if last else 0)
    bounds += [lo + round(i * (mid_hi - lo) / mid_n) for i in range(1, mid_n + 1)]
    if last:
        bounds.append(hi)
    return list(zip(bounds[:-1], bounds[1:]))


def _out_chunks():
    return _split(0, FD, N_OUT_CHUNKS, FIRST_CHUNK, LAST_CHUNK)


def _in_chunks():
    return _split(0, FD, N_IN_CHUNKS, FIRST_CHUNK, 0)


def _register_dve_op(name, spec):
    for op in dve_ops_mod.OPS:
        if op.name == name:
            return op
    opcode = dve_ops_mod._CUSTOM_DVE_ROW_BASE + len(dve_ops_mod.OPS)
    assert opcode < 0x20, "custom DVE row overflow"
    shas = {}
    for ver in ("v3", "v4"):
        uops = lower(spec, ver=ver)
        shas[ver] = DveOpSpec(
            name=name, opcode=opcode, uops=uops, rd1_en=_has_src1(spec)
        ).sha(ver)
    op = dve_ops_mod.DveOp(name, spec, subdim=False, uops_sha=shas)
    dve_ops_mod.OPS.append(op)
    dve_ops_mod._SUB_OPCODE_FOR_NAME[name] = opcode
    dve_ops_mod.CUSTOM_DVE_SPECS[name] = spec
    return op


def _get_v2_op():
    # out = (((1-t)*s0)*t + s0)*t + s1   (6 stages; s0=0.5, s1=1/6)
    body = ((One - Src0) * C0 * Src0 + C0) * Src0 + C1
    spec = Spec(
        body=body,
        reference=lambda in0, in1, s0, s1, imm2: (
            (((np.float32(1.0) - in0.astype(np.float32)) * s0) * in0 + s0)
            * in0 + s1
        ).astype(np.float32),
    )
    return _register_dve_op("BSPLINE_V2_HORNER_ANT", spec)


def _get_v0_op():
    # out = relu(in0*s0 + s1)^3          (5 ALU stages)
    r = relu(Src0 * C0 + C1)
    spec = Spec(
        body=sq(r) * r,
        reference=lambda in0, in1, s0, s1, imm2: (
            np.maximum(in0 * s0 + s1, np.float32(0.0)).astype(np.float32) ** 3
        ).astype(np.float32),
    )
    return _register_dve_op("BSPLINE_EDGE_CUBE_ANT", spec)


def _register_const(nc, value):
    """Make `value` usable as an activation bias (const_aps lookup).
    Must be called inside the TileContext: the memset is tracked by Tile."""
    f32 = mybir.dt.float32
    key = (f32, float(value))
    if key in nc.const_aps.aps:
        return
    t = nc.alloc_sbuf_tensor(f"const-f32-{float(value)}", [128, 1], f32)
    nc.vector.memset(t.ap(), float(value))
    nc.const_aps.aps[key] = t.ap()


def _build_bass():
    v2_op = _get_v2_op()
    v0_op = _get_v0_op()
    f32 = mybir.dt.float32
    f16 = mybir.dt.float16
    if SKIP_INIT_BARRIER:
        # Skip Bass.__init__'s trailing all-engine barrier (only guards its
        # 0.0/1.0 const memsets; the earlier _nrt_pseudo_barrier already
        # orders the semaphore clears).  The only in-kernel reader of those
        # consts is the throwaway table-warm activation.  Saves ~2us.
        _orig_barrier = bass.Bass.all_engine_barrier
        bass.Bass.all_engine_barrier = lambda self: None
        try:
            nc = bacc.Bacc(
                "TRN2", target_bir_lowering=False, debug=False,
                num_devices=N_CORES, enable_asserts=ENABLE_ASSERTS,
            )
        finally:
            bass.Bass.all_engine_barrier = _orig_barrier
    else:
        nc = bacc.Bacc(
            "TRN2", target_bir_lowering=False, debug=False,
            num_devices=N_CORES, enable_asserts=ENABLE_ASSERTS,
        )
    t_dram = nc.dram_tensor("t", [N_PAD], f16, kind="ExternalInput")
    out_dram = nc.dram_tensor(
        "out", [N_PLANES * N_PAD], f16, kind="ExternalOutput"
    )
    tv = t_dram.ap().rearrange("(p f) -> p f", p=P)

    with tile.TileContext(nc) as tc:
        with (
            tc.tile_pool(name="const", bufs=1) as cpool,
            tc.tile_pool(name="work", bufs=WBUFS) as wpool,
        ):
            t_tile = cpool.tile([P, FD], f16, tag="t")
            # input DMAs on the scalar HWDGE queue; outputs own the sync
            # queue so neither waits behind the other's descriptor-gen
            for lo, hi in _in_chunks():
                nc.scalar.dma_start(out=t_tile[:, lo:hi], in_=tv[:, lo:hi])

            # Warm the ACT table load before the input lands.
            warm = cpool.tile([P, 1], f32, tag="warm")
            nc.scalar.activation(
                warm[:], nc.const_aps.aps[(f32, 0.0)][:P, :],
                mybir.ActivationFunctionType.Square, bias=0.0, scale=1.0,
            )
            _register_const(nc, SQ6)   # bias for the (1-t) quadratic factor

            for ci, (lo, hi) in enumerate(_out_chunks()):
                ts = t_tile[:, lo:hi]
                w = hi - lo
                o_tile = wpool.tile([P, N_PLANES * w], f16, tag="o")
                o0, o2, o3 = o_tile[:, 0:w], o_tile[:, w:2 * w], o_tile[:, 2 * w:3 * w]

                # q3 = t^2/6 on ACT; v2 custom on DVE in parallel.
                # q3 = t^2/6 on ACT; v2 custom on DVE in parallel.
                q3 = wpool.tile([P, w], f16, tag="q3")
                nc.scalar.activation(
                    q3[:], ts, mybir.ActivationFunctionType.Square,
                    bias=0.0, scale=SQ6,
                )
                nc.vector._custom_dve(
                    v2_op, out=o2, in0=ts, s0=0.5, s1=float(1.0 / 6.0),
                )
                # v3 = (t^2/6) * t
                if GP_V3:
                    nc.gpsimd.tensor_mul(o3, q3[:], ts)
                else:
                    nc.vector.tensor_tensor(o3, q3[:], ts, mybir.AluOpType.mult)
                if V0_CUSTOM:
                    # v0 = relu(-c*t + c)^3 = (1-t)^3/6, one fused DVE op
                    nc.vector._custom_dve(
                        v0_op, out=o0, in0=ts, s0=-C1V, s1=C1V,
                    )
                else:
                    # q = (1-t); q2 = (1-t)^2/6 on ACT; v0 = q2*q
                    q = wpool.tile([P, w], f16, tag="q")
                    if Q_ENGINE == "gp":
                        nc.gpsimd.tensor_scalar(
                            q[:], ts, -1.0, 1.0,
                            mybir.AluOpType.mult, mybir.AluOpType.add,
                        )
                    elif Q_ENGINE == "dve":
                        nc.vector.tensor_scalar(
                            q[:], ts, -1.0, 1.0,
                            mybir.AluOpType.mult, mybir.AluOpType.add,
                        )
                    else:
                        nc.scalar.activation(
                            q[:], ts, mybir.ActivationFunctionType.Identity,
                            bias=1.0, scale=-1.0,
                        )
                    q2 = wpool.tile([P, w], f16, tag="q2")
                    nc.scalar.activation(
                        q2[:], ts, mybir.ActivationFunctionType.Square,
                        bias=SQ6, scale=-SQ6,
                    )
                    nc.vector.tensor_tensor(o0, q2[:], q[:], mybir.AluOpType.mult)

                ov = out_dram.ap()[N_PLANES * P * lo : N_PLANES * P * hi]
                ov = ov.rearrange("(p f) -> p f", p=P)
                nc.sync.dma_start(out=ov, in_=o_tile[:])
    nc.compile()
    return nc


_NC_CACHE = None


def _get_nc():
    global _NC_CACHE
    if _NC_CACHE is None:
        _NC_CACHE = _build_bass()
    return _NC_CACHE


def make_shards(x):
    """Host prep: t = frac(5x) as fp16 shards (one per core) + segment index
    s = floor(5x) used for host-side placement of the four value planes."""
    xf = x.astype(np.float64)
    u = 5.0 * xf
    sf = np.floor(u)
    s = sf.astype(np.int16)                  # [5, N] in 0..4
    t = (u - sf).astype(np.float16)
    in_maps = []
    for i in range(N_CORES):
        sh = np.zeros(N_PAD, dtype=np.float16)
        sh[:N_ELEM] = np.ascontiguousarray(
            t[:, i * N_SHARD : (i + 1) * N_SHARD]
        ).reshape(-1)
        in_maps.append({"t": sh})
    return in_maps, s


def _decode_planes(o_flat):
    """Undo the chunk-major [128, 3*w] packing -> [3, N_PAD] fp32."""
    planes = np.empty((N_PLANES, P, FD), dtype=np.float32)
    for lo, hi in _out_chunks():
        seg = o_flat[N_PLANES * P * lo : N_PLANES * P * hi].reshape(
            P, N_PLANES, hi - lo
        )
        planes[:, :, lo:hi] = seg.transpose(1, 0, 2)
    return planes.reshape(N_PLANES, N_PAD)


def kernel(x, grid=None, k=None, **_ignored):
    x = np.asarray(x, dtype=np.float32)
    assert x.shape == (N_ROWS, N_FULL), x.shape
    nc = _get_nc()
    in_maps, s = make_shards(x)
    res = run_bass_kernel_spmd(nc, in_maps, list(range(N_CORES))).results
    vals = np.empty((N_ROWS, 4, N_FULL), dtype=np.float32)
    for i in range(N_CORES):
        o = _decode_planes(np.asarray(res[i]["out"]))   # [3, N_PAD] f32
        sl = slice(i * N_SHARD, (i + 1) * N_SHARD)
        v = o[:, :N_ELEM].reshape(N_PLANES, N_ROWS, N_SHARD)
        vals[:, 0, sl] = v[0]
        vals[:, 2, sl] = v[1]
        vals[:, 3, sl] = v[2]
    # partition of unity: v1 = 1 - v0 - v2 - v3 (linear host epilogue)
    vals[:, 1, :] = 1.0 - vals[:, 0, :] - vals[:, 2, :] - vals[:, 3, :]
    full = np.zeros((N_ROWS, N_BASIS, N_FULL), dtype=np.float32)
    idx = (R_LO + s.astype(np.int64))[:, None, :] + np.arange(4)[None, :, None]
    np.put_along_axis(full, idx, vals, axis=1)
    return full
